# revision 14
# baseline (speedup 1.0000x reference)
"""Trainium2 Bass kernel for the SCAN-style cross-attention contrastive loss.

Sharding: image axis across 8 cores (8 images/core), captions replicated.
Each core computes its 66x8 column block of per-(caption,image) exp-sum
scores; the host gathers columns and applies the scalar hinge-loss epilogue.

Math restructure (validated against the jax reference):
  - unnormalized softmax weights u = exp(9*A_norm + wbias); the softmax
    denominator cancels in sim = num/(n1*||wctx||).
  - num  = E^T (u .* Araw)          (per-column reduction via indicator matmul)
  - q    = E^T (u .* (G_blk @ u)) = ||wctx_unnorm||^2 via per-caption Gram
  - invalid image frames are zeroed on host => their columns give e = 1
    exactly; host subtracts the known defect (F - img_len) from each exp-sum.

Performance structure:
  - all matmul operands bf16; weights padded to 128 columns (enables FWL);
    per-caption Gram blocks precomputed on host
  - ONE ACT table set (natural_log_exp_and_others, forced via the table map
    the load-insertion pass consults): Prelu = leaky-relu, Exp, and
    rsqrt(x) = exp(-0.5*ln(x)) -- zero mid-kernel table switches
  - image-frame columns are f-major (col = f*IPC + i) so the per-(word,image)
    rinv broadcast has a step-1 innermost axis -> bf16 2x DVE mode
  - per-group E-matmuls accumulate num/q into one persistent PSUM region
  - software-pipelined pair loop; engines balanced:
      ACT: lrelu, a-copy, rinv(ln+exp), exp(u)
      DVE: sq, frame-reduce, at=L*rinv, p=u*b
      GPS: q=u*a
      PE : Araw (4 chunks), b=G@u, 2x E-matmul
"""
from contextlib import ExitStack

import numpy as np
import ml_dtypes

import concourse.bacc as bacc
from concourse import hw_specs as _hw_specs
import concourse.tile as tile
from concourse import mybir
from concourse.bass_utils import run_bass_kernel_spmd

# Force every ACT instruction to resolve to the one table set that contains
# all functions we use (parametric_relu, copy, exp, ln). Set indexes are
# preserved, so the runtime id mapping stays valid; this only stops the
# load-insertion pass from ping-ponging between exp/ln anchor sets.
_JOINT_ACT_SET = "natural_log_exp_and_others"
_orig_get_tables = _hw_specs.get_activation_tables


def _forced_tables(arch):
    tabs = _orig_get_tables(arch)
    assert _JOINT_ACT_SET in tabs
    return {k: (v if k == _JOINT_ACT_SET else set()) for k, v in tabs.items()}


bacc.get_activation_tables = _forced_tables

N, F, W, D = 64, 64, 40, 512
NCORES = 8
IPC = N // NCORES        # images per core = 8
IF = IPC * F             # 512 image-frame columns per core (f-major order)
GP = 3                   # captions per partition group
NCAP = 66                # 64 captions padded to a multiple of GP
NG = NCAP // GP          # 22 groups
GW = GP * W              # 120 real partitions per group (padded to 128)
DCH = D // 128           # 4 contraction chunks
PKW = DCH * 128 + 128    # packed group width: 4x128 capT cols + 128 gram cols

f32 = mybir.dt.float32
bf16 = mybir.dt.bfloat16
FT = mybir.ActivationFunctionType
ALU = mybir.AluOpType
AX = mybir.AxisListType
BF16NP = ml_dtypes.bfloat16

MARGIN = 0.2
LAMBDA_LSE = 6.0


def _build_nc():
    nc = bacc.Bacc("TRN2", target_bir_lowering=False, debug=False)
    imgT = nc.dram_tensor("imgT", [128, DCH, IF], bf16, kind="ExternalInput").ap()
    packed = nc.dram_tensor("packed", [128, NG, PKW], bf16, kind="ExternalInput").ap()
    eall = nc.dram_tensor("eall", [128, NG, 128], bf16, kind="ExternalInput").ap()
    n1sq = nc.dram_tensor("n1sq", [NCAP, IF], f32, kind="ExternalInput").ap()
    se_out = nc.dram_tensor("se_out", [NCAP, IPC], f32, kind="ExternalOutput").ap()

    NPAIR = (NG + 1) // 2

    with tile.TileContext(nc) as tc, ExitStack() as ctx:
        const = ctx.enter_context(tc.tile_pool(name="const", bufs=1))
        pkp = ctx.enter_context(tc.tile_pool(name="pkp", bufs=6))
        lp = ctx.enter_context(tc.tile_pool(name="lp", bufs=4))
        sqp = ctx.enter_context(tc.tile_pool(name="sqp", bufs=3))
        smal = ctx.enter_context(tc.tile_pool(name="smal", bufs=3))
        up = ctx.enter_context(tc.tile_pool(name="up", bufs=3))
        pqp = ctx.enter_context(tc.tile_pool(name="pqp", bufs=3))
        epi = ctx.enter_context(tc.tile_pool(name="epi", bufs=1))
        pa = ctx.enter_context(tc.tile_pool(name="pa", bufs=2, space="PSUM"))
        pb = ctx.enter_context(tc.tile_pool(name="pb", bufs=1, space="PSUM"))
        pqn = ctx.enter_context(tc.tile_pool(name="pqn", bufs=1, space="PSUM"))

        imgT_t = const.tile([128, DCH, IF], bf16)
        nc.sync.dma_start(out=imgT_t, in_=imgT)
        eall_t = const.tile([128, NG, 128], bf16)
        nc.sync.dma_start(out=eall_t, in_=eall)
        n1sq_t = const.tile([NCAP, IF], f32)
        nc.sync.dma_start(out=n1sq_t, in_=n1sq)
        eps_col = const.tile([128, 1], f32)
        nc.vector.memset(eps_col, 1e-30)

        # persistent PSUM accumulator: [:, 0, :] = q (u^T G u), [:, 1, :] = num
        qn_ps = pqn.tile([128, 2, IF], f32)

        pk_t = [None] * NG
        L_t = [None] * NPAIR
        sq_t = [None] * NPAIR
        araw_p = [None] * NPAIR
        r2_t = [None] * NPAIR
        rv_t = [None] * NPAIR

        def dma_group(g):
            pk_t[g] = pkp.tile([128, PKW], bf16, tag="pk", name=f"pk{g}")
            nc.sync.dma_start(out=pk_t[g], in_=packed[:, g, :])

        def stage1a(k):
            """Araw matmuls for a pair of groups + paired lrelu on ACT."""
            araw_p[k] = pa.tile([128, 2, IF], f32, tag="araw", name=f"araw{k}")
            for j in range(2):
                g = 2 * k + j
                for c in range(DCH):
                    nc.tensor.matmul(out=araw_p[k][:, j, :],
                                     lhsT=pk_t[g][:, c * 128:(c + 1) * 128],
                                     rhs=imgT_t[:, c, :],
                                     start=(c == 0), stop=(c == DCH - 1))
            L_t[k] = lp.tile([128, 2, IF], bf16, tag="L", name=f"L{k}")
            nc.scalar.activation(L_t[k], araw_p[k], FT.Prelu, alpha=0.1)

        def stage1b(k):
            """sq = L*L on GPS, written i-major so the reduce is contiguous."""
            sq_t[k] = sqp.tile([128, 2, IPC, F], bf16, tag="sq", name=f"sq{k}")
            Lv = L_t[k].rearrange("p j (f i) -> p j f i", i=IPC)
            nc.gpsimd.tensor_mul(sq_t[k].rearrange("p j i f -> p j f i"), Lv, Lv)

        def stage1c(k):
            """frame-reduce r2 = sum_f L^2 (contiguous innermost f) + rinv
            = r2^-0.5 via ln+exp (same ACT table set)."""
            r2_t[k] = smal.tile([128, 2, IPC], f32, tag="r2", name=f"r2_{k}")
            nc.vector.reduce_sum(r2_t[k], sq_t[k], axis=AX.X)
            ln_t = smal.tile([128, 2, IPC], f32, tag="ln", name=f"ln{k}")
            nc.scalar.activation(ln_t, r2_t[k], FT.Ln, bias=eps_col)
            rv_t[k] = smal.tile([128, 2, IPC], bf16, tag="rv", name=f"rv{k}")
            nc.scalar.activation(rv_t[k], ln_t, FT.Exp, scale=-0.5)

        def stage2(k):
            """at = L*rinv, u = exp(9at), v = R@u, sqv = v^2, q = u*a,
            and the accumulating per-caption E-matmuls (E carries the
            invalid-word mask, R the masked Cholesky factor)."""
            at_t = up.tile([128, 2, F, IPC], bf16, tag="at")
            rvb = rv_t[k].unsqueeze(2).broadcast_to([128, 2, F, IPC])
            nc.vector.tensor_mul(
                at_t, L_t[k].rearrange("p j (f i) -> p j f i", i=IPC), rvb)
            u_t = up.tile([128, 2, IF], bf16, tag="u")
            nc.scalar.activation(u_t, at_t.rearrange("p j f i -> p j (f i)"),
                                 FT.Exp, scale=9.0)
            v_p = pb.tile([128, 2, IF], f32, tag="v")
            for j in range(2):
                nc.tensor.matmul(out=v_p[:, j, :],
                                 lhsT=pk_t[2 * k + j][:, DCH * 128:],
                                 rhs=u_t[:, j, :], start=True, stop=True)
            pq_t = pqp.tile([128, 2, 2, IF], bf16, tag="pq")
            nc.scalar.activation(pq_t[:, :, 0, :], v_p, FT.Square)
            nc.vector.tensor_mul(pq_t[:, :, 1, :], u_t, araw_p[k])
            for j in range(2):
                g = 2 * k + j
                for s in range(2):
                    nc.tensor.matmul(out=qn_ps[:, s, :], lhsT=eall_t[:, g, :],
                                     rhs=pq_t[:, j, s, :],
                                     start=(g == 0), stop=(g == NG - 1))

        # software-pipelined main loop (pairs of caption groups)
        for g in range(min(6, NG)):
            dma_group(g)
        stage1a(0)
        stage1b(0)
        stage1c(0)
        for k in range(NPAIR):
            for g in (2 * k + 6, 2 * k + 7):
                if g < NG:
                    dma_group(g)
            if k + 1 < NPAIR:
                stage1a(k + 1)
                stage1b(k + 1)
            stage2(k)
            if k + 1 < NPAIR:
                stage1c(k + 1)

        # epilogue: sim = num * (q*n1sq)^-0.5, e = exp(6 sim), sum over frames
        qs_t = epi.tile([NCAP, IF], f32)
        nc.vector.tensor_mul(qs_t, qn_ps[0:NCAP, 0, :], n1sq_t)
        lq_t = epi.tile([NCAP, IF], f32)
        nc.scalar.activation(lq_t, qs_t, FT.Ln, bias=eps_col[0:NCAP, :])
        rq_t = epi.tile([NCAP, IF], f32)
        nc.scalar.activation(rq_t, lq_t, FT.Exp, scale=-0.5)
        sim_t = epi.tile([NCAP, IF], f32)
        nc.vector.tensor_mul(sim_t, qn_ps[0:NCAP, 1, :], rq_t)
        e_t = epi.tile([NCAP, IF], f32)
        nc.scalar.activation(e_t, sim_t, FT.Exp, scale=LAMBDA_LSE)
        se_t = epi.tile([NCAP, IPC], f32)
        nc.vector.reduce_sum(se_t, e_t.rearrange("p (f i) -> p i f", i=IPC),
                             axis=AX.X)
        nc.sync.dma_start(out=se_out, in_=se_t)

    nc.compile()
    return nc


_NC = None


def _get_nc():
    global _NC
    if _NC is None:
        _NC = _build_nc()
    return _NC


def make_in_maps(images, captions, img_lens, cap_lens):
    """Host-side input preparation (numpy only): shard/transpose/mask."""
    images = np.ascontiguousarray(np.asarray(images, np.float32))
    captions = np.ascontiguousarray(np.asarray(captions, np.float32))
    img_lens = np.asarray(img_lens).astype(np.int64)
    cap_lens = np.asarray(cap_lens).astype(np.int64)

    # captions padded to 66; dummies replicate caption 0 (avoids 0/0)
    caps_p = np.concatenate(
        [captions, np.broadcast_to(captions[0:1], (NCAP - N, W, D))], axis=0)
    caps_bf = caps_p.astype(BF16NP)
    # capT [128, NG, DCH, 128]: partition = d % 128 within chunk; word columns
    # padded 120 -> 128 with zeros (weight padding for FWL)
    capT_np = np.zeros((128, NG, DCH, 128), BF16NP)
    capT_np[:, :, :, :GW] = (
        caps_bf.reshape(NG, GP, W, DCH, 128).transpose(4, 0, 3, 1, 2)
        .reshape(128, NG, DCH, GW))

    # Per-caption masked Cholesky factors: with kv = cap_len valid words,
    # G_valid = C C^T (C = valid caption rows, bf16-rounded), L_chol lower
    # with G_valid = L L^T. The kernel computes v = lhsT.T @ u = L^T u so
    # ||v||^2 = u^T G u over VALID words only -- the invalid-word mask lives
    # here and in eall, so exp needs no per-group bias.
    cf = caps_bf.astype(np.float32)
    lens_p = np.concatenate([cap_lens, np.repeat(cap_lens[0], NCAP - N)])
    chol = np.zeros((NCAP, W, W), np.float32)
    for j in range(NCAP):
        kv = int(lens_p[j])
        C = cf[j, :kv]
        Gv = (C @ C.T).astype(np.float64)
        Gv[np.diag_indices(kv)] *= 1.0 + 1e-6
        chol[j, :kv, :kv] = np.linalg.cholesky(Gv).astype(np.float32)
    gpk = np.zeros((128, NG, 128), np.float32)
    for g in range(NG):
        for b in range(GP):
            gpk[b * W:(b + 1) * W, g, b * W:(b + 1) * W] = chol[g * GP + b]
    packed_np = np.ascontiguousarray(np.concatenate(
        [capT_np.reshape(128, NG, DCH * 128), gpk.astype(BF16NP)], axis=2))

    # per-group wide indicator: group g's caption b sums into output row
    # 3g+b; rows of invalid words are zero (this applies the word mask)
    eall_np = np.zeros((128, NG, 128), np.float32)
    for g in range(NG):
        for b in range(GP):
            kv = int(lens_p[g * GP + b])
            eall_np[b * W:b * W + kv, g, g * GP + b] = 1.0
    eall_np = eall_np.astype(BF16NP)

    in_maps = []
    for core in range(NCORES):
        imgs = images[core * IPC:(core + 1) * IPC].copy()
        for i in range(IPC):
            imgs[i, img_lens[core * IPC + i]:] = 0.0
        imgs_bf = imgs.astype(BF16NP)
        # f-major frame columns: col = f*IPC + i
        Z = np.ascontiguousarray(imgs_bf.transpose(1, 0, 2).reshape(IF, D))
        imgT_np = np.ascontiguousarray(
            Z.reshape(IF, DCH, 128).transpose(2, 1, 0))  # [128, DCH, IF]
        n1 = (Z.astype(np.float32) ** 2).sum(axis=1)     # [IF], f-major
        n1sq_np = np.ascontiguousarray(
            np.broadcast_to(n1[None, :], (NCAP, IF)).astype(np.float32))
        in_maps.append({
            "imgT": imgT_np, "packed": packed_np,
            "eall": eall_np, "n1sq": n1sq_np,
        })
    return in_maps


def finish(se_list, img_lens):
    """Host epilogue: defect correction, log-sum-exp, hinge loss."""
    img_lens = np.asarray(img_lens).astype(np.int64)
    cols = []
    for core in range(NCORES):
        se = np.asarray(se_list[core], np.float32)[:N, :]         # (64, 8)
        defect = (F - img_lens[core * IPC:(core + 1) * IPC]).astype(np.float32)
        cols.append(np.log(se - defect[None, :]) / LAMBDA_LSE)
    S = np.concatenate(cols, axis=1).astype(np.float32)           # (caps, imgs)

    diag = np.diag(S)
    eye = np.eye(N, dtype=bool)
    cost_s = np.maximum(MARGIN + S - diag[:, None], 0.0)
    cost_im = np.maximum(MARGIN + S - diag[None, :], 0.0)
    cost_s[eye] = 0.0
    cost_im[eye] = 0.0
    return np.float32(cost_s.max(axis=1).sum() + cost_im.max(axis=0).sum())


def kernel(images, captions, img_lens, cap_lens):
    nc = _get_nc()
    in_maps = make_in_maps(images, captions, img_lens, cap_lens)
    res = run_bass_kernel_spmd(nc, in_maps, core_ids=list(range(NCORES)))
    se_list = [res.results[c]["se_out"] for c in range(NCORES)]
    return finish(se_list, img_lens)


# revision 16
# speedup vs baseline: 1.1173x; 1.1173x over previous
"""Trainium2 Bass kernel for the SCAN-style cross-attention contrastive loss.

Sharding: image axis across 8 cores (8 images/core), captions replicated.
Each core computes its 66x8 column block of per-(caption,image) exp-sum
scores; the host gathers columns and applies the scalar hinge-loss epilogue.

Math restructure (validated against the jax reference):
  - unnormalized softmax weights u = exp(9*A_norm + wbias); the softmax
    denominator cancels in sim = num/(n1*||wctx||).
  - num  = E^T (u .* Araw)          (per-column reduction via indicator matmul)
  - q    = E^T (u .* (G_blk @ u)) = ||wctx_unnorm||^2 via per-caption Gram
  - invalid image frames are zeroed on host => their columns give e = 1
    exactly; host subtracts the known defect (F - img_len) from each exp-sum.

Performance structure:
  - all matmul operands bf16; weights padded to 128 columns (enables FWL);
    per-caption Gram blocks precomputed on host
  - ONE ACT table set (natural_log_exp_and_others, forced via the table map
    the load-insertion pass consults): Prelu = leaky-relu, Exp, and
    rsqrt(x) = exp(-0.5*ln(x)) -- zero mid-kernel table switches
  - image-frame columns are f-major (col = f*IPC + i) so the per-(word,image)
    rinv broadcast has a step-1 innermost axis -> bf16 2x DVE mode
  - per-group E-matmuls accumulate num/q into one persistent PSUM region
  - software-pipelined pair loop; engines balanced:
      ACT: lrelu, a-copy, rinv(ln+exp), exp(u)
      DVE: sq, frame-reduce, at=L*rinv, p=u*b
      GPS: q=u*a
      PE : Araw (4 chunks), b=G@u, 2x E-matmul
"""
from contextlib import ExitStack

import numpy as np
import ml_dtypes

import concourse.bacc as bacc
from concourse import hw_specs as _hw_specs
import concourse.tile as tile
from concourse import mybir
from concourse.bass_utils import run_bass_kernel_spmd

# Force every ACT instruction to resolve to the one table set that contains
# all functions we use (parametric_relu, copy, exp, ln). Set indexes are
# preserved, so the runtime id mapping stays valid; this only stops the
# load-insertion pass from ping-ponging between exp/ln anchor sets.
_JOINT_ACT_SET = "natural_log_exp_and_others"
_orig_get_tables = _hw_specs.get_activation_tables


def _forced_tables(arch):
    tabs = _orig_get_tables(arch)
    assert _JOINT_ACT_SET in tabs
    return {k: (v if k == _JOINT_ACT_SET else set()) for k, v in tabs.items()}


bacc.get_activation_tables = _forced_tables

N, F, W, D = 64, 64, 40, 512
NCORES = 8
IPC = N // NCORES        # images per core = 8
IF = IPC * F             # 512 image-frame columns per core (f-major order)
GP = 3                   # captions per partition group
NCAP = 66                # 64 captions padded to a multiple of GP
NG = NCAP // GP          # 22 groups
GW = GP * W              # 120 real partitions per group (padded to 128)
DCH = D // 128           # 4 contraction chunks
PKW = DCH * 128 + 128    # packed group width: 4x128 capT cols + 128 gram cols

f32 = mybir.dt.float32
bf16 = mybir.dt.bfloat16
FT = mybir.ActivationFunctionType
ALU = mybir.AluOpType
AX = mybir.AxisListType
BF16NP = ml_dtypes.bfloat16

MARGIN = 0.2
LAMBDA_LSE = 6.0


def _build_nc():
    nc = bacc.Bacc("TRN2", target_bir_lowering=False, debug=False)
    imgT = nc.dram_tensor("imgT", [128, DCH, IF], bf16, kind="ExternalInput").ap()
    packed = nc.dram_tensor("packed", [128, NG, PKW], bf16, kind="ExternalInput").ap()
    eall = nc.dram_tensor("eall", [128, NG, 128], bf16, kind="ExternalInput").ap()
    n1sq = nc.dram_tensor("n1sq", [NCAP, IF], f32, kind="ExternalInput").ap()
    se_out = nc.dram_tensor("se_out", [NCAP, IPC], f32, kind="ExternalOutput").ap()

    NPAIR = (NG + 1) // 2

    with tile.TileContext(nc) as tc, ExitStack() as ctx:
        const = ctx.enter_context(tc.tile_pool(name="const", bufs=1))
        pkp = ctx.enter_context(tc.tile_pool(name="pkp", bufs=6))
        lp = ctx.enter_context(tc.tile_pool(name="lp", bufs=4))
        sqp = ctx.enter_context(tc.tile_pool(name="sqp", bufs=3))
        smal = ctx.enter_context(tc.tile_pool(name="smal", bufs=3))
        up = ctx.enter_context(tc.tile_pool(name="up", bufs=3))
        pqp = ctx.enter_context(tc.tile_pool(name="pqp", bufs=3))
        epi = ctx.enter_context(tc.tile_pool(name="epi", bufs=1))
        pa = ctx.enter_context(tc.tile_pool(name="pa", bufs=4, space="PSUM"))
        pb = ctx.enter_context(tc.tile_pool(name="pb", bufs=2, space="PSUM"))
        pqn = ctx.enter_context(tc.tile_pool(name="pqn", bufs=1, space="PSUM"))

        imgT_t = const.tile([128, DCH, IF], bf16)
        nc.sync.dma_start(out=imgT_t, in_=imgT)
        eall_t = const.tile([128, NG, 128], bf16)
        nc.sync.dma_start(out=eall_t, in_=eall)
        n1sq_t = const.tile([NCAP, IF], f32)
        nc.sync.dma_start(out=n1sq_t, in_=n1sq)
        eps_col = const.tile([128, 1], f32)
        nc.vector.memset(eps_col, 1e-30)

        # persistent PSUM accumulator: [:, 0, :] = q (u^T G u), [:, 1, :] = num
        qn_ps = pqn.tile([128, 2, IF], f32)

        pk_t = [None] * NG
        L_t = [None] * NG
        sq_t = [None] * NG
        araw_p = [None] * NG
        r2_t = [None] * NPAIR
        rv_t = [None] * NPAIR

        def dma_group(g):
            pk_t[g] = pkp.tile([128, PKW], bf16, tag="pk", name=f"pk{g}")
            nc.sync.dma_start(out=pk_t[g], in_=packed[:, g, :])

        def stage1a(g):
            """Araw matmuls + lrelu on ACT."""
            araw_p[g] = pa.tile([128, IF], f32, tag="araw", name=f"araw{g}")
            for c in range(DCH):
                nc.tensor.matmul(out=araw_p[g],
                                 lhsT=pk_t[g][:, c * 128:(c + 1) * 128],
                                 rhs=imgT_t[:, c, :],
                                 start=(c == 0), stop=(c == DCH - 1))
            L_t[g] = lp.tile([128, IF], bf16, tag="L", name=f"L{g}")
            nc.scalar.activation(L_t[g], araw_p[g], FT.Prelu, alpha=0.1)

        def stage1b(g):
            """sq = L*L on GPS (contiguous f-major layout)."""
            sq_t[g] = sqp.tile([128, IF], bf16, tag="sq", name=f"sq{g}")
            nc.gpsimd.tensor_mul(sq_t[g], L_t[g], L_t[g])

        def stage1c(g):
            """frame-reduce r2 = sum_f L^2 (strided innermost f)."""
            k, j = g // 2, g % 2
            if j == 0:
                r2_t[k] = smal.tile([128, 2, IPC], f32, tag="r2", name=f"r2_{k}")
            nc.vector.reduce_sum(r2_t[k][:, j, :],
                                 sq_t[g].rearrange("p (f i) -> p i f", i=IPC),
                                 axis=AX.X)

        def rinv(k):
            """rinv = r2^-0.5 for a pair of groups via ln+exp (one table set)."""
            ln_t = smal.tile([128, 2, IPC], f32, tag="ln", name=f"ln{k}")
            nc.scalar.activation(ln_t, r2_t[k], FT.Ln, bias=eps_col)
            rv_t[k] = smal.tile([128, 2, IPC], bf16, tag="rv", name=f"rv{k}")
            nc.scalar.activation(rv_t[k], ln_t, FT.Exp, scale=-0.5)

        def stage2(g):
            """at = L*rinv, u = exp(9at), v = R@u, sqv = v^2, q = u*a,
            and the accumulating per-caption E-matmuls (E carries the
            invalid-word mask, R the masked Cholesky factor)."""
            k, j = g // 2, g % 2
            at_t = up.tile([128, F, IPC], bf16, tag="at")
            rvb = rv_t[k][:, j, :].unsqueeze(1).broadcast_to([128, F, IPC])
            nc.vector.tensor_mul(at_t, L_t[g].rearrange("p (f i) -> p f i", i=IPC),
                                 rvb)
            u_t = up.tile([128, IF], bf16, tag="u")
            nc.scalar.activation(u_t, at_t.rearrange("p f i -> p (f i)"), FT.Exp,
                                 scale=9.0)
            v_p = pb.tile([128, IF], f32, tag="v")
            nc.tensor.matmul(out=v_p, lhsT=pk_t[g][:, DCH * 128:], rhs=u_t,
                             start=True, stop=True)
            pq_t = pqp.tile([128, 2, IF], bf16, tag="pq")
            nc.scalar.activation(pq_t[:, 0, :], v_p, FT.Square)
            nc.vector.tensor_mul(pq_t[:, 1, :], u_t, araw_p[g])
            for s in range(2):
                nc.tensor.matmul(out=qn_ps[:, s, :], lhsT=eall_t[:, g, :],
                                 rhs=pq_t[:, s, :],
                                 start=(g == 0), stop=(g == NG - 1))

        # software-pipelined main loop, stage1 runs two groups ahead
        for g in range(min(6, NG)):
            dma_group(g)
        for g in (0, 1):
            stage1a(g)
            stage1b(g)
        for g in (0, 1):
            stage1c(g)
        rinv(0)
        for g in range(NG):
            if g + 6 < NG:
                dma_group(g + 6)
            if g + 2 < NG:
                stage1a(g + 2)
                stage1b(g + 2)
            stage2(g)
            if g + 2 < NG:
                stage1c(g + 2)
            if g % 2 == 1 and (g + 1) // 2 < NPAIR:
                rinv((g + 1) // 2)

        # epilogue: sim = num * (q*n1sq)^-0.5, e = exp(6 sim), sum over frames
        qs_t = epi.tile([NCAP, IF], f32)
        nc.vector.tensor_mul(qs_t, qn_ps[0:NCAP, 0, :], n1sq_t)
        lq_t = epi.tile([NCAP, IF], f32)
        nc.scalar.activation(lq_t, qs_t, FT.Ln, bias=eps_col[0:NCAP, :])
        rq_t = epi.tile([NCAP, IF], f32)
        nc.scalar.activation(rq_t, lq_t, FT.Exp, scale=-0.5)
        sim_t = epi.tile([NCAP, IF], f32)
        nc.vector.tensor_mul(sim_t, qn_ps[0:NCAP, 1, :], rq_t)
        e_t = epi.tile([NCAP, IF], f32)
        nc.scalar.activation(e_t, sim_t, FT.Exp, scale=LAMBDA_LSE)
        se_t = epi.tile([NCAP, IPC], f32)
        nc.vector.reduce_sum(se_t, e_t.rearrange("p (f i) -> p i f", i=IPC),
                             axis=AX.X)
        nc.sync.dma_start(out=se_out, in_=se_t)

    nc.compile()
    return nc


_NC = None


def _get_nc():
    global _NC
    if _NC is None:
        _NC = _build_nc()
    return _NC


def make_in_maps(images, captions, img_lens, cap_lens):
    """Host-side input preparation (numpy only): shard/transpose/mask."""
    images = np.ascontiguousarray(np.asarray(images, np.float32))
    captions = np.ascontiguousarray(np.asarray(captions, np.float32))
    img_lens = np.asarray(img_lens).astype(np.int64)
    cap_lens = np.asarray(cap_lens).astype(np.int64)

    # captions padded to 66; dummies replicate caption 0 (avoids 0/0)
    caps_p = np.concatenate(
        [captions, np.broadcast_to(captions[0:1], (NCAP - N, W, D))], axis=0)
    caps_bf = caps_p.astype(BF16NP)
    # capT [128, NG, DCH, 128]: partition = d % 128 within chunk; word columns
    # padded 120 -> 128 with zeros (weight padding for FWL)
    capT_np = np.zeros((128, NG, DCH, 128), BF16NP)
    capT_np[:, :, :, :GW] = (
        caps_bf.reshape(NG, GP, W, DCH, 128).transpose(4, 0, 3, 1, 2)
        .reshape(128, NG, DCH, GW))

    # Per-caption masked Cholesky factors: with kv = cap_len valid words,
    # G_valid = C C^T (C = valid caption rows, bf16-rounded), L_chol lower
    # with G_valid = L L^T. The kernel computes v = lhsT.T @ u = L^T u so
    # ||v||^2 = u^T G u over VALID words only -- the invalid-word mask lives
    # here and in eall, so exp needs no per-group bias.
    cf = caps_bf.astype(np.float32)
    lens_p = np.concatenate([cap_lens, np.repeat(cap_lens[0], NCAP - N)])
    chol = np.zeros((NCAP, W, W), np.float32)
    for j in range(NCAP):
        kv = int(lens_p[j])
        C = cf[j, :kv]
        Gv = (C @ C.T).astype(np.float64)
        Gv[np.diag_indices(kv)] *= 1.0 + 1e-6
        chol[j, :kv, :kv] = np.linalg.cholesky(Gv).astype(np.float32)
    gpk = np.zeros((128, NG, 128), np.float32)
    for g in range(NG):
        for b in range(GP):
            gpk[b * W:(b + 1) * W, g, b * W:(b + 1) * W] = chol[g * GP + b]
    packed_np = np.ascontiguousarray(np.concatenate(
        [capT_np.reshape(128, NG, DCH * 128), gpk.astype(BF16NP)], axis=2))

    # per-group wide indicator: group g's caption b sums into output row
    # 3g+b; rows of invalid words are zero (this applies the word mask)
    eall_np = np.zeros((128, NG, 128), np.float32)
    for g in range(NG):
        for b in range(GP):
            kv = int(lens_p[g * GP + b])
            eall_np[b * W:b * W + kv, g, g * GP + b] = 1.0
    eall_np = eall_np.astype(BF16NP)

    in_maps = []
    for core in range(NCORES):
        imgs = images[core * IPC:(core + 1) * IPC].copy()
        for i in range(IPC):
            imgs[i, img_lens[core * IPC + i]:] = 0.0
        imgs_bf = imgs.astype(BF16NP)
        # f-major frame columns: col = f*IPC + i
        Z = np.ascontiguousarray(imgs_bf.transpose(1, 0, 2).reshape(IF, D))
        imgT_np = np.ascontiguousarray(
            Z.reshape(IF, DCH, 128).transpose(2, 1, 0))  # [128, DCH, IF]
        n1 = (Z.astype(np.float32) ** 2).sum(axis=1)     # [IF], f-major
        n1sq_np = np.ascontiguousarray(
            np.broadcast_to(n1[None, :], (NCAP, IF)).astype(np.float32))
        in_maps.append({
            "imgT": imgT_np, "packed": packed_np,
            "eall": eall_np, "n1sq": n1sq_np,
        })
    return in_maps


def finish(se_list, img_lens):
    """Host epilogue: defect correction, log-sum-exp, hinge loss."""
    img_lens = np.asarray(img_lens).astype(np.int64)
    cols = []
    for core in range(NCORES):
        se = np.asarray(se_list[core], np.float32)[:N, :]         # (64, 8)
        defect = (F - img_lens[core * IPC:(core + 1) * IPC]).astype(np.float32)
        cols.append(np.log(se - defect[None, :]) / LAMBDA_LSE)
    S = np.concatenate(cols, axis=1).astype(np.float32)           # (caps, imgs)

    diag = np.diag(S)
    eye = np.eye(N, dtype=bool)
    cost_s = np.maximum(MARGIN + S - diag[:, None], 0.0)
    cost_im = np.maximum(MARGIN + S - diag[None, :], 0.0)
    cost_s[eye] = 0.0
    cost_im[eye] = 0.0
    return np.float32(cost_s.max(axis=1).sum() + cost_im.max(axis=0).sum())


def kernel(images, captions, img_lens, cap_lens):
    nc = _get_nc()
    in_maps = make_in_maps(images, captions, img_lens, cap_lens)
    res = run_bass_kernel_spmd(nc, in_maps, core_ids=list(range(NCORES)))
    se_list = [res.results[c]["se_out"] for c in range(NCORES)]
    return finish(se_list, img_lens)


# revision 18
# speedup vs baseline: 1.3908x; 1.2448x over previous
"""Trainium2 Bass kernel for the SCAN-style cross-attention contrastive loss.

Sharding: image axis across 8 cores (8 images/core), captions replicated.
Each core computes its 66x8 column block of per-(caption,image) exp-sum
scores; the host gathers columns and applies the scalar hinge-loss epilogue.

Math restructure (validated against the jax reference):
  - unnormalized softmax weights u = exp(9*A_norm + wbias); the softmax
    denominator cancels in sim = num/(n1*||wctx||).
  - num  = E^T (u .* Araw)          (per-column reduction via indicator matmul)
  - q    = E^T (u .* (G_blk @ u)) = ||wctx_unnorm||^2 via per-caption Gram
  - invalid image frames are zeroed on host => their columns give e = 1
    exactly; host subtracts the known defect (F - img_len) from each exp-sum.

Performance structure:
  - all matmul operands bf16; weights padded to 128 columns (enables FWL);
    per-caption Gram blocks precomputed on host
  - ONE ACT table set (natural_log_exp_and_others, forced via the table map
    the load-insertion pass consults): Prelu = leaky-relu, Exp, and
    rsqrt(x) = exp(-0.5*ln(x)) -- zero mid-kernel table switches
  - image-frame columns are f-major (col = f*IPC + i) so the per-(word,image)
    rinv broadcast has a step-1 innermost axis -> bf16 2x DVE mode
  - per-group E-matmuls accumulate num/q into one persistent PSUM region
  - software-pipelined pair loop; engines balanced:
      ACT: lrelu, a-copy, rinv(ln+exp), exp(u)
      DVE: sq, frame-reduce, at=L*rinv, p=u*b
      GPS: q=u*a
      PE : Araw (4 chunks), b=G@u, 2x E-matmul
"""
from contextlib import ExitStack

import numpy as np
import ml_dtypes

import concourse.bacc as bacc
from concourse import hw_specs as _hw_specs
import concourse.tile as tile
from concourse import mybir
from concourse.bass_utils import run_bass_kernel_spmd

# Force every ACT instruction to resolve to the one table set that contains
# all functions we use (parametric_relu, copy, exp, ln). Set indexes are
# preserved, so the runtime id mapping stays valid; this only stops the
# load-insertion pass from ping-ponging between exp/ln anchor sets.
_JOINT_ACT_SET = "natural_log_exp_and_others"
_orig_get_tables = _hw_specs.get_activation_tables


def _forced_tables(arch):
    tabs = _orig_get_tables(arch)
    assert _JOINT_ACT_SET in tabs
    return {k: (v if k == _JOINT_ACT_SET else set()) for k, v in tabs.items()}


bacc.get_activation_tables = _forced_tables

N, F, W, D = 64, 64, 40, 512
NCORES = 8
IPC = N // NCORES        # images per core = 8
IF = IPC * F             # 512 image-frame columns per core (f-major order)
GP = 3                   # captions per partition group
NCAP = 66                # 64 captions padded to a multiple of GP
NG = NCAP // GP          # 22 groups
GW = GP * W              # 120 real partitions per group (padded to 128)
DCH = D // 128           # 4 contraction chunks
PKW = DCH * 128 + 128    # packed group width: 4x128 capT cols + 128 gram cols

f32 = mybir.dt.float32
bf16 = mybir.dt.bfloat16
FT = mybir.ActivationFunctionType
ALU = mybir.AluOpType
AX = mybir.AxisListType
BF16NP = ml_dtypes.bfloat16

MARGIN = 0.2
LAMBDA_LSE = 6.0


def _build_nc():
    nc = bacc.Bacc("TRN2", target_bir_lowering=False, debug=False)
    imgT = nc.dram_tensor("imgT", [128, DCH, IF], bf16, kind="ExternalInput").ap()
    packed = nc.dram_tensor("packed", [128, NG, PKW], bf16, kind="ExternalInput").ap()
    eall = nc.dram_tensor("eall", [128, NG, 128], bf16, kind="ExternalInput").ap()
    n1sq = nc.dram_tensor("n1sq", [NCAP, IF], f32, kind="ExternalInput").ap()
    se_out = nc.dram_tensor("se_out", [NCAP, IPC], f32, kind="ExternalOutput").ap()

    NPAIR = (NG + 1) // 2

    with tile.TileContext(nc) as tc, ExitStack() as ctx:
        const = ctx.enter_context(tc.tile_pool(name="const", bufs=1))
        pkp = ctx.enter_context(tc.tile_pool(name="pkp", bufs=6))
        lp = ctx.enter_context(tc.tile_pool(name="lp", bufs=4))
        sqp = ctx.enter_context(tc.tile_pool(name="sqp", bufs=3))
        smal = ctx.enter_context(tc.tile_pool(name="smal", bufs=3))
        up = ctx.enter_context(tc.tile_pool(name="up", bufs=3))
        pqp = ctx.enter_context(tc.tile_pool(name="pqp", bufs=3))
        epi = ctx.enter_context(tc.tile_pool(name="epi", bufs=1))
        pa = ctx.enter_context(tc.tile_pool(name="pa", bufs=2, space="PSUM"))
        pb = ctx.enter_context(tc.tile_pool(name="pb", bufs=1, space="PSUM"))
        pqn = ctx.enter_context(tc.tile_pool(name="pqn", bufs=1, space="PSUM"))

        imgT_t = const.tile([128, DCH, IF], bf16)
        nc.sync.dma_start(out=imgT_t, in_=imgT)
        eall_t = const.tile([128, NG, 128], bf16)
        nc.sync.dma_start(out=eall_t, in_=eall)
        n1sq_t = const.tile([NCAP, IF], f32)
        nc.sync.dma_start(out=n1sq_t, in_=n1sq)
        eps_col = const.tile([128, 1], f32)
        nc.vector.memset(eps_col, 1e-30)

        # persistent PSUM accumulator: [:, 0, :] = q (u^T G u), [:, 1, :] = num
        qn_ps = pqn.tile([128, 2, IF], f32)

        pk_t = [None] * NG
        L_t = [None] * NPAIR
        araw_p = [None] * NPAIR
        rv_t = [None] * NPAIR

        def dma_group(g):
            pk_t[g] = pkp.tile([128, PKW], bf16, tag="pk", name=f"pk{g}")
            nc.sync.dma_start(out=pk_t[g], in_=packed[:, g, :])

        def stage1(k):
            """Pair: Araw matmuls, L = lrelu(a) on ACT, r2 = sum_f L^2 via a
            contiguous binary-tree reduce on DVE, rinv = r2^-0.5 via ln+exp."""
            araw_p[k] = pa.tile([128, 2, IF], f32, tag="araw", name=f"araw{k}")
            for j in range(2):
                g = 2 * k + j
                for c in range(DCH):
                    nc.tensor.matmul(out=araw_p[k][:, j, :],
                                     lhsT=pk_t[g][:, c * 128:(c + 1) * 128],
                                     rhs=imgT_t[:, c, :],
                                     start=(c == 0), stop=(c == DCH - 1))
            L_t[k] = lp.tile([128, 2, IF], bf16, tag="L", name=f"L{k}")
            nc.scalar.activation(L_t[k], araw_p[k], FT.Prelu, alpha=0.1)
            sq_t = sqp.tile([128, 2, IF], bf16, tag="sq")
            nc.vector.tensor_mul(sq_t, L_t[k], L_t[k])
            # tree-reduce over frames: halves are contiguous in f-major layout
            cur = sq_t
            m = IF
            while m > 2 * IPC:
                nxt = sqp.tile([128, 2, m // 2], bf16, tag=f"tr{m}",
                               name=f"tr{m}_{k}")
                nc.vector.tensor_add(nxt, cur[:, :, 0:m // 2], cur[:, :, m // 2:m])
                cur = nxt
                m //= 2
            r2_t = smal.tile([128, 2, IPC], f32, tag="r2", name=f"r2_{k}")
            nc.vector.tensor_add(r2_t, cur[:, :, 0:IPC], cur[:, :, IPC:2 * IPC])
            ln_t = smal.tile([128, 2, IPC], f32, tag="ln", name=f"ln{k}")
            nc.scalar.activation(ln_t, r2_t, FT.Ln, bias=eps_col)
            rv_t[k] = smal.tile([128, 2, IPC], bf16, tag="rv", name=f"rv{k}")
            nc.scalar.activation(rv_t[k], ln_t, FT.Exp, scale=-0.5)

        def stage2(k):
            """at = L*rinv, u = exp(9at), v = R@u, sqv = v^2,
            q = u*a = min(t, 10t) with t = u*L, and the accumulating
            per-caption E-matmuls (E carries the invalid-word mask, R the
            masked Cholesky factor)."""
            at_t = up.tile([128, 2, F, IPC], bf16, tag="at")
            for j in range(2):
                rvb = rv_t[k][:, j, :].unsqueeze(1).broadcast_to([128, F, IPC])
                nc.vector.tensor_mul(
                    at_t[:, j, :, :],
                    L_t[k][:, j, :].rearrange("p (f i) -> p f i", i=IPC), rvb)
            u_t = up.tile([128, 2, IF], bf16, tag="u")
            nc.scalar.activation(u_t, at_t.rearrange("p j f i -> p j (f i)"),
                                 FT.Exp, scale=9.0)
            v_p = pb.tile([128, 2, IF], f32, tag="v")
            for j in range(2):
                nc.tensor.matmul(out=v_p[:, j, :],
                                 lhsT=pk_t[2 * k + j][:, DCH * 128:],
                                 rhs=u_t[:, j, :], start=True, stop=True)
            pq_t = pqp.tile([128, 2, 2, IF], bf16, tag="pq")
            nc.scalar.activation(pq_t[:, :, 0, :], v_p, FT.Square)
            t_t = up.tile([128, 2, IF], bf16, tag="t")
            nc.vector.tensor_mul(t_t, u_t, L_t[k])
            tm_t = up.tile([128, 2, IF], bf16, tag="tm")
            nc.vector.tensor_scalar(tm_t, t_t, 0.0, 9.0, ALU.min, ALU.mult)
            nc.vector.tensor_add(pq_t[:, :, 1, :], t_t, tm_t)
            for j in range(2):
                g = 2 * k + j
                for s in range(2):
                    nc.tensor.matmul(out=qn_ps[:, s, :], lhsT=eall_t[:, g, :],
                                     rhs=pq_t[:, j, s, :],
                                     start=(g == 0), stop=(g == NG - 1))

        # software-pipelined pair loop; stage1 runs two pairs ahead of stage2
        for g in range(min(8, NG)):
            dma_group(g)
        stage1(0)
        stage1(1)
        for k in range(NPAIR):
            for g in (2 * k + 8, 2 * k + 9):
                if g < NG:
                    dma_group(g)
            stage2(k)
            if k + 2 < NPAIR:
                stage1(k + 2)

        # epilogue: sim = num * (q*n1sq)^-0.5, e = exp(6 sim), sum over frames
        qs_t = epi.tile([NCAP, IF], f32)
        nc.vector.tensor_mul(qs_t, qn_ps[0:NCAP, 0, :], n1sq_t)
        lq_t = epi.tile([NCAP, IF], f32)
        nc.scalar.activation(lq_t, qs_t, FT.Ln, bias=eps_col[0:NCAP, :])
        rq_t = epi.tile([NCAP, IF], f32)
        nc.scalar.activation(rq_t, lq_t, FT.Exp, scale=-0.5)
        sim_t = epi.tile([NCAP, IF], f32)
        nc.vector.tensor_mul(sim_t, qn_ps[0:NCAP, 1, :], rq_t)
        e_t = epi.tile([NCAP, IF], f32)
        nc.scalar.activation(e_t, sim_t, FT.Exp, scale=LAMBDA_LSE)
        se_t = epi.tile([NCAP, IPC], f32)
        nc.vector.reduce_sum(se_t, e_t.rearrange("p (f i) -> p i f", i=IPC),
                             axis=AX.X)
        nc.sync.dma_start(out=se_out, in_=se_t)

    nc.compile()
    return nc


_NC = None


def _get_nc():
    global _NC
    if _NC is None:
        _NC = _build_nc()
    return _NC


def make_in_maps(images, captions, img_lens, cap_lens):
    """Host-side input preparation (numpy only): shard/transpose/mask."""
    images = np.ascontiguousarray(np.asarray(images, np.float32))
    captions = np.ascontiguousarray(np.asarray(captions, np.float32))
    img_lens = np.asarray(img_lens).astype(np.int64)
    cap_lens = np.asarray(cap_lens).astype(np.int64)

    # captions padded to 66; dummies replicate caption 0 (avoids 0/0)
    caps_p = np.concatenate(
        [captions, np.broadcast_to(captions[0:1], (NCAP - N, W, D))], axis=0)
    caps_bf = caps_p.astype(BF16NP)
    # capT [128, NG, DCH, 128]: partition = d % 128 within chunk; word columns
    # padded 120 -> 128 with zeros (weight padding for FWL)
    capT_np = np.zeros((128, NG, DCH, 128), BF16NP)
    capT_np[:, :, :, :GW] = (
        caps_bf.reshape(NG, GP, W, DCH, 128).transpose(4, 0, 3, 1, 2)
        .reshape(128, NG, DCH, GW))

    # Per-caption masked Cholesky factors: with kv = cap_len valid words,
    # G_valid = C C^T (C = valid caption rows, bf16-rounded), L_chol lower
    # with G_valid = L L^T. The kernel computes v = lhsT.T @ u = L^T u so
    # ||v||^2 = u^T G u over VALID words only -- the invalid-word mask lives
    # here and in eall, so exp needs no per-group bias.
    cf = caps_bf.astype(np.float32)
    lens_p = np.concatenate([cap_lens, np.repeat(cap_lens[0], NCAP - N)])
    chol = np.zeros((NCAP, W, W), np.float32)
    for j in range(NCAP):
        kv = int(lens_p[j])
        C = cf[j, :kv]
        Gv = (C @ C.T).astype(np.float64)
        Gv[np.diag_indices(kv)] *= 1.0 + 1e-6
        chol[j, :kv, :kv] = np.linalg.cholesky(Gv).astype(np.float32)
    gpk = np.zeros((128, NG, 128), np.float32)
    for g in range(NG):
        for b in range(GP):
            gpk[b * W:(b + 1) * W, g, b * W:(b + 1) * W] = chol[g * GP + b]
    packed_np = np.ascontiguousarray(np.concatenate(
        [capT_np.reshape(128, NG, DCH * 128), gpk.astype(BF16NP)], axis=2))

    # per-group wide indicator: group g's caption b sums into output row
    # 3g+b; rows of invalid words are zero (this applies the word mask)
    eall_np = np.zeros((128, NG, 128), np.float32)
    for g in range(NG):
        for b in range(GP):
            kv = int(lens_p[g * GP + b])
            eall_np[b * W:b * W + kv, g, g * GP + b] = 1.0
    eall_np = eall_np.astype(BF16NP)

    in_maps = []
    for core in range(NCORES):
        imgs = images[core * IPC:(core + 1) * IPC].copy()
        for i in range(IPC):
            imgs[i, img_lens[core * IPC + i]:] = 0.0
        imgs_bf = imgs.astype(BF16NP)
        # f-major frame columns: col = f*IPC + i
        Z = np.ascontiguousarray(imgs_bf.transpose(1, 0, 2).reshape(IF, D))
        imgT_np = np.ascontiguousarray(
            Z.reshape(IF, DCH, 128).transpose(2, 1, 0))  # [128, DCH, IF]
        n1 = (Z.astype(np.float32) ** 2).sum(axis=1)     # [IF], f-major
        n1sq_np = np.ascontiguousarray(
            np.broadcast_to(n1[None, :], (NCAP, IF)).astype(np.float32))
        in_maps.append({
            "imgT": imgT_np, "packed": packed_np,
            "eall": eall_np, "n1sq": n1sq_np,
        })
    return in_maps


def finish(se_list, img_lens):
    """Host epilogue: defect correction, log-sum-exp, hinge loss."""
    img_lens = np.asarray(img_lens).astype(np.int64)
    cols = []
    for core in range(NCORES):
        se = np.asarray(se_list[core], np.float32)[:N, :]         # (64, 8)
        defect = (F - img_lens[core * IPC:(core + 1) * IPC]).astype(np.float32)
        cols.append(np.log(se - defect[None, :]) / LAMBDA_LSE)
    S = np.concatenate(cols, axis=1).astype(np.float32)           # (caps, imgs)

    diag = np.diag(S)
    eye = np.eye(N, dtype=bool)
    cost_s = np.maximum(MARGIN + S - diag[:, None], 0.0)
    cost_im = np.maximum(MARGIN + S - diag[None, :], 0.0)
    cost_s[eye] = 0.0
    cost_im[eye] = 0.0
    return np.float32(cost_s.max(axis=1).sum() + cost_im.max(axis=0).sum())


def kernel(images, captions, img_lens, cap_lens):
    nc = _get_nc()
    in_maps = make_in_maps(images, captions, img_lens, cap_lens)
    res = run_bass_kernel_spmd(nc, in_maps, core_ids=list(range(NCORES)))
    se_list = [res.results[c]["se_out"] for c in range(NCORES)]
    return finish(se_list, img_lens)
